# revision 34
# baseline (speedup 1.0000x reference)
"""Non-local block (self-attention over 64x64 spatial map) on 8 NeuronCores.

Sharding: data-parallel over batch (B=8 -> 1 image per core). Each core runs
the full N=4096 attention for its image; no collectives.

Per-core layout strategy:
  - theta_x/phi_x stored [O=96, N=4096] fp16; scores computed TRANSPOSED
    (S^T[m, q] chunks) so softmax denominators come out of the PE itself via
    an appended ones-column on the PV lhsT.
  - exp(S - 45) on ScalarE in [128, 1024] chunks (constant shift cancels
    exactly in softmax; bigger chunks amortize the ACT access bubble).
  - PV: lhsT = [g^T | ones] [128m, 97], rhs = expS^T [128m, 512q]
    -> y_u [97, 512] accumulated over 32 m-chunks; row 96 = softmax sums.
  - normalization commutes with the linear out-projection: 1/l is broadcast
    across partitions with a single-pass float32r PE outer product, applied
    after W^T y_u, off the critical path (tails software-pipelined one
    q-super behind the matmul stream).
"""

import numpy as np
import ml_dtypes

B, C, O = 8, 192, 96
HH, WW = 64, 64
N = HH * WW           # 4096
NQ = 8                # q-supers of 512
QS = 512
NMC = N // 128        # 32 m-chunks
N_CORES = 8

_CACHE = {}


def _build():
    from contextlib import ExitStack
    import concourse.tile as tile
    from concourse import bacc, mybir

    dt = mybir.dt
    AF = mybir.ActivationFunctionType

    nc = bacc.Bacc("TRN2", target_bir_lowering=False, debug=False,
                   num_devices=N_CORES)

    x_d = nc.dram_tensor("x", [C, N], dt.float32, kind="ExternalInput").ap()
    xh_d = nc.dram_tensor("xh", [C, N], dt.float16, kind="ExternalInput").ap()
    wt_d = {}
    b_d = {}
    for p in ("theta", "phi", "g"):
        wt_d[p] = nc.dram_tensor(f"wt_{p}", [C, O], dt.float16,
                                 kind="ExternalInput").ap()
        b_d[p] = nc.dram_tensor(f"b_{p}", [O, 1], dt.float32,
                                kind="ExternalInput").ap()
    wWT_d = nc.dram_tensor("w_WT", [O, C], dt.bfloat16, kind="ExternalInput").ap()
    bW_d = nc.dram_tensor("b_W", [C, 1], dt.float32, kind="ExternalInput").ap()
    out_d = nc.dram_tensor("out", [C, N], dt.float32, kind="ExternalOutput").ap()

    with tile.TileContext(nc) as tc:
        with ExitStack() as ctx:
            # ---------------- SBUF pools ----------------
            consts = ctx.enter_context(tc.tile_pool(name="consts", bufs=1))
            xpool = ctx.enter_context(tc.tile_pool(name="x", bufs=1))
            acts = ctx.enter_context(tc.tile_pool(name="acts", bufs=1))
            expp = ctx.enter_context(tc.tile_pool(name="exp", bufs=1))
            ypool = ctx.enter_context(tc.tile_pool(name="y", bufs=3))
            outp = ctx.enter_context(tc.tile_pool(name="outsb", bufs=3))
            # ---------------- PSUM pools (shared by all phases) ----------
            ps_qk = ctx.enter_context(
                tc.tile_pool(name="ps_qk", bufs=2, space="PSUM"))
            ps_pv = ctx.enter_context(
                tc.tile_pool(name="ps_pv", bufs=2, space="PSUM"))
            ps_aux = ctx.enter_context(
                tc.tile_pool(name="ps_aux", bufs=2, space="PSUM"))

            wt = {}
            bias = {}
            for p in ("theta", "phi", "g"):
                wt[p] = consts.tile([96, 2 * O], dt.float16, tag=f"wt_{p}",
                                    name=f"wt_{p}")
                nc.gpsimd.dma_start(wt[p][:, 0:O], wt_d[p][0:96, :])
                nc.gpsimd.dma_start(wt[p][:, O:2 * O], wt_d[p][96:192, :])
                bias[p] = consts.tile([O, 1], dt.float32, tag=f"b_{p}",
                                      name=f"b_{p}")
                nc.gpsimd.dma_start(bias[p][:], b_d[p][:])
            wWT = consts.tile([O, C], dt.bfloat16, tag="wWT")
            nc.gpsimd.dma_start(wWT[:], wWT_d[:])
            bW = [consts.tile([96, 1], dt.float32, tag=f"bW{h}", name=f"bW{h}")
                  for h in (0, 1)]
            for h in (0, 1):
                nc.gpsimd.dma_start(bW[h][:], bW_d[96 * h:96 * h + 96, :])

            cneg45 = consts.tile([128, 1], dt.float32, tag="cneg45")
            nc.vector.memset(cneg45[:], -45.0)

            theta_sb = acts.tile([O, N], dt.float16, tag="theta")
            phi_sb = acts.tile([O, N], dt.float16, tag="phi")
            gt_ones = acts.tile([128, 97 * NMC], dt.bfloat16, tag="gt")
            nc.vector.memset(gt_ones[:], 1.0)
            expS = expp.tile([128, NMC * QS], dt.bfloat16, tag="expS")

            # x halves: fp16 for matmul (first: it gates everything),
            # fp32 for the residual
            xh = [xpool.tile([96, N], dt.float16, tag=f"xh{h}", name=f"xh{h}")
                  for h in (0, 1)]
            xf = [xpool.tile([96, N], dt.float32, tag=f"xf{h}", name=f"xf{h}")
                  for h in (0, 1)]
            for j in range(NQ):
                cs = slice(j * QS, (j + 1) * QS)
                for h in (0, 1):
                    nc.sync.dma_start(xh[h][:, cs], xh_d[96 * h:96 * h + 96, cs])
            for j in range(NQ):
                cs = slice(j * QS, (j + 1) * QS)
                for h in (0, 1):
                    nc.gpsimd.dma_start(xf[h][:, cs], x_d[96 * h:96 * h + 96, cs])
            for h in (0, 1):
                nc.vector.tensor_scalar_add(xf[h][:], xf[h][:], bW[h][:])

            # ---------------- P1 + P2, software-pipelined ----------------
            # window w streams QK(w)+exp(w) on ACT while PE also runs
            # PV(w-1), tail(w-2), and dribbles leftover projection work.
            # g^T is produced directly by the PE (lhsT = x chunk), its bias
            # is folded into b_W on the host.

            def emit_proj_mms(p, j, h, state):
                if "ps" not in state:
                    state["ps"] = ps_aux.tile([128, QS], dt.float32,
                                              tag="aux", name=f"proj_{p}_{j}")
                ps = state["ps"]
                cs = slice(j * QS, (j + 1) * QS)
                nc.tensor.matmul(ps[0:O, :],
                                 wt[p][:, 96 * h:96 * h + O],
                                 xh[h][:, cs],
                                 start=(h == 0), stop=(h == 1))
                if h == 1:
                    dst = {"phi": phi_sb, "theta": theta_sb}[p]
                    nc.vector.tensor_scalar_add(dst[:, cs], ps[0:O, :],
                                                bias[p][:])

            def emit_gt(mc):
                # gt chunk mc via lhsT = x chunk (direct g^T)
                ps = ps_aux.tile([128, QS], dt.float32, tag="aux",
                                 name=f"gt_{mc}")
                mcs = slice(128 * mc, 128 * mc + 128)
                nc.tensor.matmul(ps[:, 0:96], xh[0][:, mcs],
                                 wt["g"][:, 0:O], start=True, stop=False)
                nc.tensor.matmul(ps[:, 0:96], xh[1][:, mcs],
                                 wt["g"][:, O:2 * O], start=False, stop=True)
                nc.vector.tensor_copy(gt_ones[:, 97 * mc:97 * mc + 96],
                                      ps[:, 0:96])

            def proj_items(p, pr):
                items = []
                for k in (0, 1):
                    state = {}
                    items += [
                        lambda j=2 * pr + k, h=h, s=state:
                        emit_proj_mms(p, j, h, s) for h in (0, 1)]
                return items

            def emit_qk_exp(qs, pr):
                ps = ps_qk.tile([128, 2 * QS], dt.float32, tag="qk",
                                name=f"qk_{qs}_{pr}")
                qcols = slice(qs * QS, (qs + 1) * QS)
                for k in (0, 1):
                    mc = 2 * pr + k
                    nc.tensor.matmul(
                        ps[:, k * QS:(k + 1) * QS],
                        phi_sb[:, 128 * mc:128 * mc + 128],
                        theta_sb[:, qcols], start=True, stop=True)
                nc.scalar.activation(
                    expS[:, 1024 * pr:1024 * pr + 1024], ps[:],
                    AF.Exp, bias=cneg45[:])

            def emit_pv(qs, mc, ypsum):
                nc.tensor.matmul(
                    ypsum[:], gt_ones[:, 97 * mc:97 * mc + 97],
                    expS[:, 512 * mc:512 * mc + 512],
                    start=(mc == 0), stop=(mc == NMC - 1))

            tails = {}

            def tail_step(qs, pr, ypsum):
                if pr == 1:
                    y_bf = ypool.tile([O, QS], dt.bfloat16, tag="ybf",
                                      name=f"ybf_{qs}")
                    nc.vector.tensor_copy(y_bf[:], ypsum[0:96, :])
                    linv_sb = ypool.tile([1, QS], dt.float32, tag="linv",
                                         name=f"linv_{qs}")
                    nc.vector.reciprocal(linv_sb[:], ypsum[96:97, :])
                    li_sb = ypool.tile([O, QS], dt.float32, tag="lisb",
                                       name=f"lisb_{qs}")
                    nc.gpsimd.partition_broadcast(li_sb[:], linv_sb[:])
                    tails[qs] = (y_bf, li_sb, None)
                elif pr in (2, 3) and qs in tails:
                    h = pr - 2
                    y_bf, li_sb = tails[qs][0], tails[qs][1]
                    qcols = slice(qs * QS, (qs + 1) * QS)
                    pso = ps_aux.tile([128, QS], dt.float32, tag="aux",
                                      name=f"pso_{qs}_{h}")
                    nc.tensor.matmul(pso[0:96, :],
                                     wWT[:, 96 * h:96 * h + 96],
                                     y_bf[:], start=True, stop=True)
                    ob = outp.tile([96, QS], dt.float32, tag="ob",
                                   name=f"ob_{qs}_{h}")
                    nc.vector.tensor_mul(ob[:], pso[0:96, :], li_sb[:])
                    nc.vector.tensor_add(ob[:], ob[:], xf[h][:, qcols])
                    nc.sync.dma_start(out_d[96 * h:96 * h + 96, qcols], ob[:])

            for j in (0, 1):
                st = {}
                for h in (0, 1):
                    emit_proj_mms("phi", j, h, st)
            for j in (0, 1):
                st = {}
                for h in (0, 1):
                    emit_proj_mms("theta", j, h, st)

            work = []
            for pr in (1, 2, 3):
                work += [(1.0, f) for f in proj_items("phi", pr)]
            work += [(0.5, lambda mc=mc: emit_gt(mc)) for mc in range(NMC)]
            for pr in (1, 2, 3):
                work += [(1.0, f) for f in proj_items("theta", pr)]

            ypsums = {}
            for w in range(NQ):
                if w >= 1:
                    ypsums[w - 1] = ps_pv.tile([97, QS], dt.float32, tag="pv",
                                               name=f"pv_{w - 1}")
                for pr in range(NMC // 2):
                    if not (w == 0 and pr == 0):
                        budget = 1.0
                        while work and budget >= work[0][0]:
                            budget -= work[0][0]
                            work.pop(0)[1]()
                    if w >= 1:
                        for k in (0, 1):
                            emit_pv(w - 1, 2 * pr + k, ypsums[w - 1])
                    emit_qk_exp(w, pr)
                    if w >= 2:
                        tail_step(w - 2, pr, ypsums[w - 2])
            # epilogue: PV(7), tail(6), tail(7)
            ypsums[NQ - 1] = ps_pv.tile([97, QS], dt.float32, tag="pv",
                                        name=f"pv_{NQ - 1}")
            for pr in range(NMC // 2):
                for k in (0, 1):
                    emit_pv(NQ - 1, 2 * pr + k, ypsums[NQ - 1])
                tail_step(NQ - 2, pr, ypsums[NQ - 2])
            for pr in (1, 2, 3):
                tail_step(NQ - 1, pr, ypsums[NQ - 1])
            while work:
                work.pop(0)[1]()

    nc.compile()
    return nc


def _get_nc():
    if "nc" not in _CACHE:
        _CACHE["nc"] = _build()
    return _CACHE["nc"]


LAST_RESULTS = None


def kernel(x, g_w, g_b, theta_w, theta_b, phi_w, phi_b, W_w, W_b):
    global LAST_RESULTS
    from concourse.bass_utils import run_bass_kernel_spmd

    nc = _get_nc()

    x = np.asarray(x, dtype=np.float32)
    common = {
        "wt_theta": np.ascontiguousarray(np.asarray(theta_w).T).astype(np.float16),
        "wt_phi": np.ascontiguousarray(np.asarray(phi_w).T).astype(np.float16),
        "wt_g": np.ascontiguousarray(np.asarray(g_w).T).astype(np.float16),
        "w_WT": np.ascontiguousarray(np.asarray(W_w).T).astype(ml_dtypes.bfloat16),
        "b_theta": np.asarray(theta_b, dtype=np.float32).reshape(O, 1),
        "b_phi": np.asarray(phi_b, dtype=np.float32).reshape(O, 1),
        "b_g": np.asarray(g_b, dtype=np.float32).reshape(O, 1),
        "b_W": (np.asarray(W_b, dtype=np.float32)
                + np.asarray(W_w, dtype=np.float32)
                @ np.asarray(g_b, dtype=np.float32)).reshape(C, 1),
    }
    in_maps = []
    for b in range(B):
        xb = np.ascontiguousarray(x[b].reshape(C, N))
        in_maps.append({"x": xb, "xh": xb.astype(np.float16), **common})
    res = run_bass_kernel_spmd(nc, in_maps, list(range(N_CORES)))
    LAST_RESULTS = res
    out = np.stack([res.results[b]["out"].reshape(C, HH, WW) for b in range(B)])
    return out.astype(np.float32)
